# revision 5
# baseline (speedup 1.0000x reference)
"""Trainium2 kernel for nn_EnhancedHeterogeneousLoss (segment_reduce).

Strategy (8-core SPMD, data-parallel over the node dim):
  - batch is sorted, so the host splits the 1M nodes at segment boundaries:
    core c owns segments [512c, 512(c+1)) and exactly the nodes that belong
    to them.  Each core's range is split into 8 chunks of 64 segments; the
    nodes of one chunk (<= 16384 of them) are padded to exactly 16384 and
    shipped as one contiguous [128, 128*67] fp16 supertile (2.2 MB DMA).
  - On device, each 128-node tile contributes via a one-hot matmul:
        psum[67, 64] += x_tile[128, 67].T @ onehot[128, 64]
    where onehot[i, j] = (batch_local[i] == j) is built with one DVE
    tensor_scalar(is_equal) against a constant iota row (fp16 -> 4x mode).
    The 67th feature column is a host-appended column of ones, so segment
    counts fall out of the same matmul.  PSUM accumulates fp32 over the 128
    tiles of a chunk; padded nodes carry sentinel id -1 and match nothing.
  - Per-core output is the [67, 512] block of (feat_sums | counts)^T for its
    own segments; the host concatenates the 8 disjoint blocks and finishes
    the tiny per-graph math (O(B*F)) on the CPU in float64.
"""

import numpy as np

P = 128                       # partitions / nodes per matmul tile
F = 66                        # node feature dim
FE = F + 1                    # features + count column
B = 4096                      # number of graphs / segments
N_CORES = 8
NSEG_CHUNK = 64               # segments per chunk (= matmul free dim N)
CHUNKS_PER_CORE = 8
SEGS_PER_CORE = NSEG_CHUNK * CHUNKS_PER_CORE      # 512
TS = 128                      # node-tiles per chunk
CHUNK_NODES = TS * P          # 16384 padded nodes per chunk (one DMA)
XW = TS * FE                  # chunk supertile free width: 8576
EPS = 1e-8
ALPHA, BETA, GAMMA, DELTA = 0.4, 0.35, 0.2, 0.05

_NC_CACHE = None


def _dt():
    import concourse.mybir as mybir

    return mybir.dt.float16, np.float16


def _build_bass():
    """Build + compile the per-core Bass program (same program on all cores)."""
    import concourse.mybir as mybir
    import concourse.tile as tile
    from concourse import bacc

    dt_x, _ = _dt()
    nc = bacc.Bacc(
        "TRN2", target_bir_lowering=False, debug=False, num_devices=N_CORES
    )
    x_d = nc.dram_tensor(
        "xd", [CHUNKS_PER_CORE, P, XW], dt_x, kind="ExternalInput"
    )
    # scalar operand of tensor_scalar(is_equal) must be fp32
    bf_d = nc.dram_tensor(
        "bf", [CHUNKS_PER_CORE, P, TS], mybir.dt.float32, kind="ExternalInput"
    )
    iota_d = nc.dram_tensor("iota", [P, NSEG_CHUNK], dt_x, kind="ExternalInput")
    out_d = nc.dram_tensor(
        "out", [FE, SEGS_PER_CORE], mybir.dt.float32, kind="ExternalOutput"
    )

    with tile.TileContext(nc) as tc:
        with (
            tc.tile_pool(name="xs", bufs=3) as xpool,
            tc.tile_pool(name="oh", bufs=6) as ohpool,
            tc.tile_pool(name="bfp", bufs=2) as bfpool,
            tc.tile_pool(name="misc", bufs=1) as misc,
            tc.tile_pool(name="ps", bufs=2, space="PSUM") as pspool,
        ):
            iota_t = misc.tile([P, NSEG_CHUNK], dt_x)
            nc.sync.dma_start(out=iota_t[:], in_=iota_d[:])
            outbuf = misc.tile([FE, SEGS_PER_CORE], mybir.dt.float32)
            for k in range(CHUNKS_PER_CORE):
                bf_t = bfpool.tile([P, TS], mybir.dt.float32)
                nc.sync.dma_start(out=bf_t[:], in_=bf_d[k])
                xs = xpool.tile([P, XW], dt_x)
                nc.sync.dma_start(out=xs[:], in_=x_d[k])
                ps = pspool.tile([FE, NSEG_CHUNK], mybir.dt.float32)
                for t in range(TS):
                    oh = ohpool.tile([P, NSEG_CHUNK], dt_x)
                    nc.vector.tensor_scalar(
                        out=oh[:],
                        in0=iota_t[:],
                        scalar1=bf_t[:, t : t + 1],
                        scalar2=None,
                        op0=mybir.AluOpType.is_equal,
                    )
                    nc.tensor.matmul(
                        out=ps[:],
                        lhsT=xs[:, t * FE : (t + 1) * FE],
                        rhs=oh[:],
                        start=(t == 0),
                        stop=(t == TS - 1),
                    )
                nc.vector.tensor_copy(
                    out=outbuf[:, k * NSEG_CHUNK : (k + 1) * NSEG_CHUNK], in_=ps[:]
                )
            nc.sync.dma_start(out=out_d[:], in_=outbuf[:])
    nc.compile()
    return nc


def _get_nc():
    global _NC_CACHE
    if _NC_CACHE is None:
        _NC_CACHE = _build_bass()
    return _NC_CACHE


def _shard_inputs(x, batch):
    """Split nodes at segment boundaries into 64 chunks, pad each chunk to
    CHUNK_NODES, and lay the data out exactly as the device consumes it."""
    _, np_x = _dt()
    batch = np.asarray(batch)
    n_chunks = N_CORES * CHUNKS_PER_CORE
    off = np.searchsorted(batch, np.arange(0, B + 1, NSEG_CHUNK))
    x_maps = np.zeros((N_CORES, CHUNKS_PER_CORE, P, XW), dtype=np_x)
    bf_maps = np.full((N_CORES, CHUNKS_PER_CORE, P, TS), -1.0, dtype=np.float32)
    xa = np.zeros((CHUNK_NODES, FE), dtype=np_x)
    ba = np.empty(CHUNK_NODES, dtype=np.float32)
    for j in range(n_chunks):
        c, k = divmod(j, CHUNKS_PER_CORE)
        lo, hi = int(off[j]), int(off[j + 1])
        n = hi - lo
        assert n <= CHUNK_NODES, f"chunk {j} has {n} nodes > {CHUNK_NODES}"
        xa[:] = 0.0
        xa[:n, :F] = x[lo:hi]
        xa[:n, F] = 1.0
        ba[:] = -1.0
        ba[:n] = batch[lo:hi] - j * NSEG_CHUNK
        # node (t, p) of the chunk -> xd[k, p, t*FE:(t+1)*FE]
        x_maps[c, k] = xa.reshape(TS, P, FE).transpose(1, 0, 2).reshape(P, XW)
        bf_maps[c, k] = ba.reshape(TS, P).T
    return x_maps, bf_maps


def _make_in_maps(x, batch):
    _, np_x = _dt()
    x_maps, bf_maps = _shard_inputs(x, batch)
    iota = np.broadcast_to(
        np.arange(NSEG_CHUNK, dtype=np_x), (P, NSEG_CHUNK)
    ).copy()
    return [
        {"xd": x_maps[c], "bf": bf_maps[c], "iota": iota} for c in range(N_CORES)
    ]


def _segment_stats_device(x, batch):
    """Run the 8-core bass kernel; returns (feat_sums [B, F], counts [B])."""
    from concourse.bass_utils import run_bass_kernel_spmd

    nc = _get_nc()
    in_maps = _make_in_maps(x, batch)
    res = run_bass_kernel_spmd(nc, in_maps, list(range(N_CORES)))
    fsT = np.concatenate(
        [res.results[c]["out"] for c in range(N_CORES)], axis=1
    )  # [FE, B]
    feat_sums = fsT[:F].T.astype(np.float64)  # [B, F]
    counts = fsT[F].astype(np.float64)  # [B]
    return feat_sums, counts


def _final_loss(predictions, targets, feat_sums, counts, device_weights):
    """Replicates the per-graph tail of the reference in float64."""
    logits = np.asarray(predictions, dtype=np.float64).reshape(-1)
    tf = np.asarray(targets, dtype=np.float64)
    dw = np.asarray(device_weights, dtype=np.float64)

    bce = np.maximum(logits, 0.0) - logits * tf + np.log1p(np.exp(-np.abs(logits)))
    probs = 1.0 / (1.0 + np.exp(-logits))
    uncertainty = 1.0 - np.abs(probs - 0.5) * 2.0
    base_loss = np.mean(bce * (1.0 + 2.0 * uncertainty))

    with np.errstate(invalid="ignore", divide="ignore"):
        dev_sums = feat_sums[:, :34] + EPS
        device_dist = dev_sums / np.sum(dev_sums, axis=-1, keepdims=True)
        err = np.abs(probs - tf)
        device_importance = device_dist @ dw
        entropy = -np.sum(device_dist * np.log(device_dist + EPS), axis=-1)
        per_graph_dev = err * device_importance + entropy * err * 0.15
        valid_dev = counts > 0
        n_dev = max(np.sum(valid_dev.astype(np.float64)), 1.0)
        device_loss = np.sum(np.where(valid_dev, per_graph_dev, 0.0)) / n_dev

        safe_counts = np.maximum(counts, 1.0)[:, None]
        means = feat_sums / safe_counts
        drnl = means[:, :32]
        hde = means[:, F - 32 : F]
        dot = np.sum(drnl * hde, axis=-1)
        na = np.maximum(np.linalg.norm(drnl, axis=-1), EPS)
        nb = np.maximum(np.linalg.norm(hde, axis=-1), EPS)
        feature_sim = np.abs(dot / (na * nb))
        pred_conf = np.abs(probs - 0.5) * 2.0
        consistency = np.abs(feature_sim - pred_conf) * 1.2
        valid_topo = counts >= 2
        n_topo = max(np.sum(valid_topo.astype(np.float64)), 1.0)
        topology_loss = np.sum(np.where(valid_topo, consistency, 0.0)) / n_topo

    pos = (tf == 1.0).astype(np.float64)
    neg = (tf == 0.0).astype(np.float64)
    npos, nneg = np.sum(pos), np.sum(neg)
    pos_loss = np.sum(np.maximum(0.6 - probs, 0.0) * pos) / max(npos, 1.0)
    neg_loss = np.sum(np.maximum(probs - 0.4, 0.0) * neg) / max(nneg, 1.0)
    contrast_loss = (pos_loss + neg_loss) if (npos > 0 and nneg > 0) else 0.0

    total = (
        ALPHA * base_loss
        + BETA * device_loss
        + GAMMA * topology_loss
        + DELTA * contrast_loss
    )
    return np.array(total, dtype=np.float32)


def kernel(predictions, targets, x, batch, device_weights):
    feat_sums, counts = _segment_stats_device(x, batch)
    return _final_loss(predictions, targets, feat_sums, counts, device_weights)
